# revision 1
# baseline (speedup 1.0000x reference)
"""Trainium2 Bass kernel for nn_NEURAL_PYSCF_WF (neural wavefunction).

reference:
  mo   = einsum('ben,mn->bem', ao, mo_weight)          # [B, 32, 128]
  sub  = mo[:, cfg[:,:,None], cfg[:,None,:]]           # [B, 128, 16, 16]
  dets = det(sub)                                      # [B, 128]
  out  = dets @ ci_weight.T                            # [B, 1]

Since config indices are < 32, only mo[:, :, :32] matters.

Strategy (8 NeuronCores, data-parallel over B=8192):
  per core (1024 batch rows, 8 tiles of 128):
   phase 1: load ao tiles [128,128] -> PE transpose -> aoT; matmul with
            W32T stationary -> M^T [m, (b,e)] in PSUM -> SBUF -> DRAM
            scratch laid out [32, BC*32]; reload per b-tile as
            M [128b, 1024(e,m)] via strided DMA.
   phase 2: GPSIMD ap_gather pulls the 16x16 submatrices for a chunk of
            configs into [128b, Ct*256]; DVE runs a pivot-free LU
            (clamped pivots, tau=1e-6) batched over (b on partitions,
            configs along free dim); det = prod(pivots); weighted sum
            over configs with ci -> out[b].
"""

from contextlib import ExitStack

import numpy as np

import concourse.bass as bass
import concourse.bacc as bacc
import concourse.mybir as mybir
import concourse.tile as tile
from concourse.bass_utils import run_bass_kernel_spmd

F32 = mybir.dt.float32
I16 = mybir.dt.int16
AX = mybir.AxisListType
OP = mybir.AluOpType

B = 8192
NE = 32      # electrons (and the max config index)
NAO = 128
K = 16       # config size
NCONF = 128
NCORES = 8
BC = B // NCORES
TAU = 1e-6


def build_gidx(cfg: np.ndarray) -> np.ndarray:
    """Wrapped int16 gather indices for ap_gather, [128, NCONF*K*K/16]."""
    c, k = cfg.shape
    assert k == K
    idx = (cfg[:, :, None].astype(np.int64) * NE + cfg[:, None, :]).reshape(-1)
    assert idx.max() < NE * NE
    n = idx.shape[0]
    wrapped = idx.reshape(n // 16, 16).T.astype(np.int16)  # [16, n/16]
    return np.tile(wrapped, (8, 1))


def emit_program(nc, tc, aps, BCc: int, Ct: int):
    ctx = ExitStack()
    NBT = BCc // 128
    NCH = NCONF // Ct
    ao, w32t, ident, cirep, gidx, mscr, out = (
        aps["ao"], aps["w32t"], aps["ident"], aps["cirep"], aps["gidx"],
        aps["mscr"], aps["out"])

    with ctx:
        cpool = ctx.enter_context(tc.tile_pool(name="consts", bufs=1))
        nat = ctx.enter_context(tc.tile_pool(name="nat", bufs=4))
        tp_ps = ctx.enter_context(
            tc.tile_pool(name="tp_ps", bufs=3, space="PSUM"))
        aot = ctx.enter_context(tc.tile_pool(name="aot", bufs=2))
        m_ps = ctx.enter_context(
            tc.tile_pool(name="m_ps", bufs=3, space="PSUM"))
        msb = ctx.enter_context(tc.tile_pool(name="msb", bufs=2))
        sub = ctx.enter_context(tc.tile_pool(name="sub", bufs=2))
        lb = ctx.enter_context(tc.tile_pool(name="lb", bufs=2))
        pb = ctx.enter_context(tc.tile_pool(name="pb", bufs=2))
        sm = ctx.enter_context(tc.tile_pool(name="sm", bufs=8))
        dets = ctx.enter_context(tc.tile_pool(name="dets", bufs=2))
        outp = ctx.enter_context(tc.tile_pool(name="outp", bufs=1))

        w32t_s = cpool.tile([128, NE], F32)
        ident_s = cpool.tile([128, 128], F32)
        cirep_s = cpool.tile([128, NCONF], F32)
        gidx_s = cpool.tile([128, NCONF * K * K // 16], I16)
        nc.sync.dma_start(w32t_s[:], w32t[:])
        nc.sync.dma_start(ident_s[:], ident[:])
        nc.sync.dma_start(cirep_s[:], cirep[:])
        nc.sync.dma_start(gidx_s[:], gidx[:])

        out_sb = outp.tile([128, NBT], F32)

        ao3 = ao.rearrange("(t p) n -> t p n", p=128)
        # mscr: [32m, BC*32(b,e)] — M^T layout
        mscr_r = mscr.rearrange("m (b e) -> b m e", e=NE)   # [BC, 32, 32]

        for bt in range(NBT):
            # ------------- phase 1: M^T = W32 @ ao^T -------------------
            aot_full = aot.tile([128, 32 * 128], F32)
            for t in range(32):
                nat_t = nat.tile([128, 128], F32)
                nc.sync.dma_start(nat_t[:], ao3[bt * 32 + t])
                ps = tp_ps.tile([128, 128], F32)
                nc.tensor.transpose(ps[:], nat_t[:], ident_s[:])
                nc.scalar.copy(aot_full[:, t * 128:(t + 1) * 128], ps[:])
            for t in range(32):
                mp = m_ps.tile([NE, 128], F32)
                nc.tensor.matmul(
                    mp[:], w32t_s[:], aot_full[:, t * 128:(t + 1) * 128],
                    start=True, stop=True)
                msb_s = nat.tile([NE, 128], F32, tag="mstage")
                nc.scalar.copy(msb_s[:], mp[:])
                nc.scalar.dma_start(
                    mscr[:, (bt * 128 + t * 4) * NE:
                         (bt * 128 + t * 4 + 4) * NE],
                    msb_s[:])

            # ------------- phase 2: dets -------------------------------
            msb_t = msb.tile([128, NE * NE], F32)
            nc.sync.dma_start(msb_t[:], mscr_r[bt * 128:(bt + 1) * 128])
            dets_t = dets.tile([128, NCONF], F32)
            for ch in range(NCH):
                sub_t = sub.tile([128, Ct * K * K], F32)
                nc.gpsimd.ap_gather(
                    sub_t[:], msb_t[:],
                    gidx_s[:, ch * Ct * K * K // 16:
                           (ch + 1) * Ct * K * K // 16],
                    channels=128, num_elems=NE * NE, d=1, num_idxs=Ct * K * K)
                det_acc = dets_t[:, ch * Ct:(ch + 1) * Ct]
                nc.vector.memset(det_acc, 1.0)

                S4 = sub_t[:].rearrange("p (c i j) -> p c i j", i=K, j=K)
                lbuf = lb.tile([128, Ct * (K - 1)], F32)
                L3 = lbuf[:].rearrange("p (c i) -> p c i", c=Ct)
                pbuf = pb.tile([128, Ct * (K - 1) * (K - 1)], F32)
                P4 = pbuf[:].rearrange(
                    "p (c i j) -> p c i j", i=K - 1, j=K - 1)

                for k in range(K):
                    r = K - 1 - k
                    piv = S4[:, :, k, k]
                    sgn = sm.tile([128, Ct], F32, tag="sgn")
                    psafe = sm.tile([128, Ct], F32, tag="psafe")
                    nc.vector.tensor_scalar(
                        sgn[:], piv, 0.0, None, op0=OP.is_ge)
                    nc.vector.tensor_scalar(
                        sgn[:], sgn[:], 2.0 * TAU, -TAU,
                        op0=OP.mult, op1=OP.add)
                    nc.vector.tensor_tensor(
                        psafe[:], piv, sgn[:], op=OP.add)
                    nc.vector.tensor_tensor(
                        det_acc, det_acc, psafe[:], op=OP.mult)
                    if r == 0:
                        continue
                    mrecip = sm.tile([128, Ct], F32, tag="mrecip")
                    nc.vector.reciprocal(mrecip[:], psafe[:])
                    nc.vector.tensor_scalar_mul(mrecip[:], mrecip[:], -1.0)
                    col = S4[:, :, k + 1:, k]
                    row = S4[:, :, k, k + 1:]
                    Lv = L3[:, :, :r]
                    nc.vector.tensor_tensor(
                        Lv, col,
                        mrecip[:].unsqueeze(2).broadcast_to([128, Ct, r]),
                        op=OP.mult)
                    Pv = P4[:, :, :r, :r]
                    nc.vector.tensor_tensor(
                        Pv,
                        Lv.unsqueeze(3).broadcast_to([128, Ct, r, r]),
                        row.unsqueeze(2).broadcast_to([128, Ct, r, r]),
                        op=OP.mult)
                    Sv = S4[:, :, k + 1:, k + 1:]
                    nc.vector.tensor_tensor(Sv, Sv, Pv, op=OP.add)

            wd = sub.tile([128, NCONF], F32, tag="wd")
            nc.vector.tensor_tensor(wd[:], dets_t[:], cirep_s[:], op=OP.mult)
            nc.vector.tensor_reduce(
                out_sb[:, bt:bt + 1], wd[:], axis=AX.X, op=OP.add)

        nc.sync.dma_start(out[:], out_sb[:])


def build(BCc: int, Ct: int = 32):
    nc = bacc.Bacc("TRN2", target_bir_lowering=False, debug=False)
    aps = {}
    aps["ao"] = nc.dram_tensor(
        "ao", [BCc * NE, NAO], F32, kind="ExternalInput").ap()
    aps["w32t"] = nc.dram_tensor(
        "w32t", [NAO, NE], F32, kind="ExternalInput").ap()
    aps["ident"] = nc.dram_tensor(
        "ident", [128, 128], F32, kind="ExternalInput").ap()
    aps["cirep"] = nc.dram_tensor(
        "cirep", [128, NCONF], F32, kind="ExternalInput").ap()
    aps["gidx"] = nc.dram_tensor(
        "gidx", [128, NCONF * K * K // 16], I16, kind="ExternalInput").ap()
    aps["mscr"] = nc.dram_tensor("mscr", [NE, BCc * NE], F32).ap()
    aps["out"] = nc.dram_tensor(
        "out", [128, BCc // 128], F32, kind="ExternalOutput").ap()

    with tile.TileContext(nc) as tc:
        emit_program(nc, tc, aps, BCc, Ct)
    nc.compile()
    return nc


def host_inputs(ao_shard, mo_weight, ci_weight, configs):
    BCc = ao_shard.shape[0]
    w32 = mo_weight[:NE, :]
    return {
        "ao": np.ascontiguousarray(
            ao_shard.reshape(BCc * NE, NAO)).astype(np.float32),
        "w32t": np.ascontiguousarray(w32.T).astype(np.float32),
        "ident": np.eye(128, dtype=np.float32),
        "cirep": np.ascontiguousarray(
            np.tile(ci_weight.astype(np.float32), (128, 1))),
        "gidx": build_gidx(configs),
    }


_CACHE: dict = {}


def _get_program():
    key = ("prog", BC)
    if key not in _CACHE:
        _CACHE[key] = build(BC)
    return _CACHE[key]


def kernel(ao, mo_weight, ci_weight, configs, _trace=False, _trace_kwargs=None):
    ao = np.asarray(ao, dtype=np.float32)
    mo_weight = np.asarray(mo_weight, dtype=np.float32)
    ci_weight = np.asarray(ci_weight, dtype=np.float32)
    configs = np.asarray(configs, dtype=np.int32)
    assert ao.shape == (B, NE, NAO)

    nc = _get_program()
    in_maps = [
        host_inputs(ao[c * BC:(c + 1) * BC], mo_weight, ci_weight, configs)
        for c in range(NCORES)
    ]
    res = run_bass_kernel_spmd(
        nc, in_maps, core_ids=list(range(NCORES)),
        trace=_trace, **(_trace_kwargs or {}))
    outs = []
    for c in range(NCORES):
        o = np.asarray(res.results[c]["out"])      # [128, NBT]
        outs.append(o.T.reshape(-1))               # b = bt*128 + p
    full = np.concatenate(outs).astype(np.float32)[:, None]
    if _trace:
        return full, res
    return full


def ref_algo(ao_shard, mo_weight, ci_weight, configs):
    """Numpy replica of the on-device algorithm (dev checking only)."""
    M = np.einsum("ben,mn->bem", ao_shard, mo_weight[:NE]).astype(np.float32)
    sub = M[:, configs[:, :, None], configs[:, None, :]].astype(np.float32)
    Bs = sub.shape[0]
    A = sub.reshape(-1, K, K).copy()
    det = np.ones(A.shape[0], dtype=np.float32)
    tau = np.float32(TAU)
    for k in range(K):
        piv = A[:, k, k].copy()
        s = np.where(piv >= 0, np.float32(1), np.float32(-1))
        ps = piv + s * tau
        det *= ps
        rec = (np.float32(1.0) / ps).astype(np.float32)
        L = (-A[:, k + 1:, k] * rec[:, None]).astype(np.float32)
        A[:, k + 1:, k + 1:] += (
            L[:, :, None] * A[:, None, k, k + 1:].reshape(A.shape[0], 1, -1)
        ).astype(np.float32)
    dets = det.reshape(Bs, NCONF)
    return (dets @ ci_weight.T.astype(np.float32)).astype(np.float32)


# revision 2
# speedup vs baseline: 1.5436x; 1.5436x over previous
"""Trainium2 Bass kernel for nn_NEURAL_PYSCF_WF (neural wavefunction).

reference:
  mo   = einsum('ben,mn->bem', ao, mo_weight)          # [B, 32, 128]
  sub  = mo[:, cfg[:,:,None], cfg[:,None,:]]           # [B, 128, 16, 16]
  dets = det(sub)                                      # [B, 128]
  out  = dets @ ci_weight.T                            # [B, 1]

Config indices are < 32, so only mo[:, :, :32] matters.

Strategy (8 NeuronCores, data-parallel over B=8192). Per core (1024 rows):
  phase 1: ao tiles -> PE transpose -> matmul (W32T stationary) ->
           M^T [m,(b,e)] -> DRAM scratch; reload per 128-row b-tile as
           M [128b, 1024(e,m)].
  phase 2 per chunk of Ct configs:
           gather1 (GPSIMD ap_gather, d=16): row-halves of each config's
             16 rows -> R [b, c, i, m32]
           transpose copy (DVE, strided): R -> Rt [b, c, m32, i16]
           gather2 (d=16): config columns -> subT [b, c, j, i]
           pivot-free LU on DVE batched over (b partitions, configs in
           free dim); reciprocal clamped to +-1e6; det = prod(diag) via
           product tree (det(A^T) == det(A)).
  out[b] = sum_c ci[c] * det[b, c]  (TT mult + reduce).
"""

from contextlib import ExitStack

import numpy as np

import concourse.bass as bass
import concourse.bacc as bacc
import concourse.mybir as mybir
import concourse.tile as tile
from concourse.bass_utils import run_bass_kernel_spmd

F32 = mybir.dt.float32
I16 = mybir.dt.int16
AX = mybir.AxisListType
OP = mybir.AluOpType

B = 8192
NE = 32      # electrons (and the max config index)
NAO = 128
K = 16       # config size
NCONF = 128
NCORES = 8
BC = B // NCORES
RCLAMP = 1e6
CT = 16      # configs per chunk


def wrap_idx(idx: np.ndarray) -> np.ndarray:
    """Wrap a flat index list into ap_gather's [128, n/16] layout."""
    n = idx.shape[0]
    assert n % 16 == 0
    w = idx.reshape(n // 16, 16).T.astype(np.int16)
    return np.tile(w, (8, 1))


def build_gidx1(cfg: np.ndarray) -> np.ndarray:
    """Stage-1 indices: (c, i, h) -> block cfg[c,i]*2 + h (d=16 units)."""
    c = cfg.shape[0]
    idx = (cfg[:, :, None].astype(np.int64) * 2
           + np.arange(2)[None, None, :]).reshape(-1)
    return wrap_idx(idx)          # [128, NCONF*32/16]


def build_gidx2(cfg: np.ndarray) -> np.ndarray:
    """Stage-2 indices per chunk: (c_local, j) -> c_local*32 + cfg[c,j]."""
    nch = NCONF // CT
    cols = []
    for ch in range(nch):
        sl = cfg[ch * CT:(ch + 1) * CT]                    # [CT, 16]
        idx = (np.arange(CT)[:, None] * NE + sl).reshape(-1)
        cols.append(wrap_idx(idx))                         # [128, CT]
    return np.concatenate(cols, axis=1)                    # [128, NCONF]


def emit_program(nc, tc, aps, BCc: int):
    ctx = ExitStack()
    NBT = BCc // 128
    NCH = NCONF // CT
    ao, w32t, ident, cirep, gidx1, gidx2, mscr, out = (
        aps["ao"], aps["w32t"], aps["ident"], aps["cirep"], aps["gidx1"],
        aps["gidx2"], aps["mscr"], aps["out"])

    with ctx:
        cpool = ctx.enter_context(tc.tile_pool(name="consts", bufs=1))
        nat = ctx.enter_context(tc.tile_pool(name="nat", bufs=4))
        tp_ps = ctx.enter_context(
            tc.tile_pool(name="tp_ps", bufs=3, space="PSUM"))
        aot = ctx.enter_context(tc.tile_pool(name="aot", bufs=2))
        m_ps = ctx.enter_context(
            tc.tile_pool(name="m_ps", bufs=3, space="PSUM"))
        msb = ctx.enter_context(tc.tile_pool(name="msb", bufs=2))
        rp = ctx.enter_context(tc.tile_pool(name="rp", bufs=1))
        rtp = ctx.enter_context(tc.tile_pool(name="rtp", bufs=1))
        sub = ctx.enter_context(tc.tile_pool(name="sub", bufs=2))
        lb = ctx.enter_context(tc.tile_pool(name="lb", bufs=2))
        pb = ctx.enter_context(tc.tile_pool(name="pb", bufs=1))
        sm = ctx.enter_context(tc.tile_pool(name="sm", bufs=8))
        dets = ctx.enter_context(tc.tile_pool(name="dets", bufs=2))
        outp = ctx.enter_context(tc.tile_pool(name="outp", bufs=1))

        w32t_s = cpool.tile([128, NE], F32)
        ident_s = cpool.tile([128, 128], F32)
        cirep_s = cpool.tile([128, NCONF], F32)
        gidx1_s = cpool.tile([128, NCONF * 2], I16)
        gidx2_s = cpool.tile([128, NCONF], I16)
        nc.sync.dma_start(w32t_s[:], w32t[:])
        nc.sync.dma_start(ident_s[:], ident[:])
        nc.sync.dma_start(cirep_s[:], cirep[:])
        nc.sync.dma_start(gidx1_s[:], gidx1[:])
        nc.sync.dma_start(gidx2_s[:], gidx2[:])

        out_sb = outp.tile([128, NBT], F32)

        ao3 = ao.rearrange("(t p) n -> t p n", p=128)
        # mscr: [32m, BC*32(b,e)] — M^T layout
        mscr_r = mscr.rearrange("m (b e) -> b m e", e=NE)   # [BC, 32, 32]

        for bt in range(NBT):
            # ------------- phase 1: M^T = W32 @ ao^T -------------------
            aot_full = aot.tile([128, 32 * 128], F32)
            for t in range(32):
                nat_t = nat.tile([128, 128], F32)
                nc.sync.dma_start(nat_t[:], ao3[bt * 32 + t])
                ps = tp_ps.tile([128, 128], F32)
                nc.tensor.transpose(ps[:], nat_t[:], ident_s[:])
                nc.scalar.copy(aot_full[:, t * 128:(t + 1) * 128], ps[:])
            for t in range(32):
                mp = m_ps.tile([NE, 128], F32)
                nc.tensor.matmul(
                    mp[:], w32t_s[:], aot_full[:, t * 128:(t + 1) * 128],
                    start=True, stop=True)
                msb_s = nat.tile([NE, 128], F32, tag="mstage")
                nc.scalar.copy(msb_s[:], mp[:])
                nc.scalar.dma_start(
                    mscr[:, (bt * 128 + t * 4) * NE:
                         (bt * 128 + t * 4 + 4) * NE],
                    msb_s[:])

            # ------------- phase 2: dets -------------------------------
            msb_t = msb.tile([128, NE * NE], F32)
            nc.sync.dma_start(msb_t[:], mscr_r[bt * 128:(bt + 1) * 128])
            dets_t = dets.tile([128, NCONF], F32)
            for ch in range(NCH):
                # stage 1: rows (c,i) as two 16-elem halves
                r_t = rp.tile([128, CT * K * NE], F32)
                nc.gpsimd.ap_gather(
                    r_t[:], msb_t[:],
                    gidx1_s[:, ch * CT * 2:(ch + 1) * CT * 2],
                    channels=128, num_elems=NE * 2, d=16,
                    num_idxs=CT * K * 2)
                # transpose [c, i, m] -> [c, m, i]
                rt_t = rtp.tile([128, CT * K * NE], F32)
                rt_dst = bass.AP(
                    rt_t[:].tensor, rt_t[:].offset,
                    [[int(rt_t[:].ap[0][0]), 128],
                     [K * NE, CT], [K, NE], [1, K]])
                r_src = bass.AP(
                    r_t[:].tensor, r_t[:].offset,
                    [[int(r_t[:].ap[0][0]), 128],
                     [K * NE, CT], [1, NE], [NE, K]])
                nc.vector.tensor_copy(rt_dst, r_src)
                # stage 2: columns cfg[c,j] -> subT [c, j, i]
                sub_t = sub.tile([128, CT * K * K], F32)
                nc.gpsimd.ap_gather(
                    sub_t[:], rt_t[:],
                    gidx2_s[:, ch * CT:(ch + 1) * CT],
                    channels=128, num_elems=CT * NE, d=16, num_idxs=CT * K)

                # ---- pivot-free LU over [c, 16, 16] (transposed sub) ----
                S4 = sub_t[:].rearrange("p (c i j) -> p c i j", i=K, j=K)
                lbuf = lb.tile([128, CT * (K - 1)], F32)
                L3 = lbuf[:].rearrange("p (c i) -> p c i", c=CT)
                pbuf = pb.tile([128, CT * (K - 1) * (K - 1)], F32)
                P4 = pbuf[:].rearrange(
                    "p (c i j) -> p c i j", i=K - 1, j=K - 1)

                for k in range(K - 1):
                    r = K - 1 - k
                    piv = S4[:, :, k, k]
                    rec = sm.tile([128, CT], F32, tag="rec")
                    nc.vector.reciprocal(rec[:], piv)
                    nc.vector.tensor_scalar(
                        rec[:], rec[:], -RCLAMP, RCLAMP,
                        op0=OP.max, op1=OP.min)
                    col = S4[:, :, k + 1:, k]
                    row = S4[:, :, k, k + 1:]
                    Lv = L3[:, :, :r]
                    nc.vector.tensor_tensor(
                        Lv, col,
                        rec[:].unsqueeze(2).broadcast_to([128, CT, r]),
                        op=OP.mult)
                    Pv = P4[:, :, :r, :r]
                    nc.vector.tensor_tensor(
                        Pv,
                        Lv.unsqueeze(3).broadcast_to([128, CT, r, r]),
                        row.unsqueeze(2).broadcast_to([128, CT, r, r]),
                        op=OP.mult)
                    Sv = S4[:, :, k + 1:, k + 1:]
                    nc.vector.tensor_tensor(Sv, Sv, Pv, op=OP.subtract)

                # det = prod(diag) via product tree
                diag = sub_t[:]
                t8 = sm.tile([128, CT * 8], F32, tag="t8")
                nc.vector.tensor_tensor(
                    t8[:].rearrange("p (c x) -> p c x", c=CT),
                    bass.AP(diag.tensor, diag.offset,
                            [[int(diag.ap[0][0]), 128], [K * K, CT], [34, 8]]),
                    bass.AP(diag.tensor, diag.offset + 17,
                            [[int(diag.ap[0][0]), 128], [K * K, CT], [34, 8]]),
                    op=OP.mult)
                t4 = sm.tile([128, CT * 4], F32, tag="t4")
                nc.vector.tensor_tensor(
                    t4[:].rearrange("p (c x) -> p c x", c=CT),
                    bass.AP(t8[:].tensor, t8[:].offset,
                            [[int(t8[:].ap[0][0]), 128], [8, CT], [2, 4]]),
                    bass.AP(t8[:].tensor, t8[:].offset + 1,
                            [[int(t8[:].ap[0][0]), 128], [8, CT], [2, 4]]),
                    op=OP.mult)
                t2 = sm.tile([128, CT * 2], F32, tag="t2")
                nc.vector.tensor_tensor(
                    t2[:].rearrange("p (c x) -> p c x", c=CT),
                    bass.AP(t4[:].tensor, t4[:].offset,
                            [[int(t4[:].ap[0][0]), 128], [4, CT], [2, 2]]),
                    bass.AP(t4[:].tensor, t4[:].offset + 1,
                            [[int(t4[:].ap[0][0]), 128], [4, CT], [2, 2]]),
                    op=OP.mult)
                nc.vector.tensor_tensor(
                    dets_t[:, ch * CT:(ch + 1) * CT],
                    bass.AP(t2[:].tensor, t2[:].offset,
                            [[int(t2[:].ap[0][0]), 128], [2, CT]]),
                    bass.AP(t2[:].tensor, t2[:].offset + 1,
                            [[int(t2[:].ap[0][0]), 128], [2, CT]]),
                    op=OP.mult)

            wd = sub.tile([128, NCONF], F32, tag="wd")
            nc.vector.tensor_tensor(wd[:], dets_t[:], cirep_s[:], op=OP.mult)
            nc.vector.tensor_reduce(
                out_sb[:, bt:bt + 1], wd[:], axis=AX.X, op=OP.add)

        nc.sync.dma_start(out[:], out_sb[:])


def build(BCc: int):
    nc = bacc.Bacc("TRN2", target_bir_lowering=False, debug=False)
    aps = {}
    aps["ao"] = nc.dram_tensor(
        "ao", [BCc * NE, NAO], F32, kind="ExternalInput").ap()
    aps["w32t"] = nc.dram_tensor(
        "w32t", [NAO, NE], F32, kind="ExternalInput").ap()
    aps["ident"] = nc.dram_tensor(
        "ident", [128, 128], F32, kind="ExternalInput").ap()
    aps["cirep"] = nc.dram_tensor(
        "cirep", [128, NCONF], F32, kind="ExternalInput").ap()
    aps["gidx1"] = nc.dram_tensor(
        "gidx1", [128, NCONF * 2], I16, kind="ExternalInput").ap()
    aps["gidx2"] = nc.dram_tensor(
        "gidx2", [128, NCONF], I16, kind="ExternalInput").ap()
    aps["mscr"] = nc.dram_tensor("mscr", [NE, BCc * NE], F32).ap()
    aps["out"] = nc.dram_tensor(
        "out", [128, BCc // 128], F32, kind="ExternalOutput").ap()

    with tile.TileContext(nc) as tc:
        emit_program(nc, tc, aps, BCc)
    nc.compile()
    return nc


def host_inputs(ao_shard, mo_weight, ci_weight, configs):
    BCc = ao_shard.shape[0]
    w32 = mo_weight[:NE, :]
    return {
        "ao": np.ascontiguousarray(
            ao_shard.reshape(BCc * NE, NAO)).astype(np.float32),
        "w32t": np.ascontiguousarray(w32.T).astype(np.float32),
        "ident": np.eye(128, dtype=np.float32),
        "cirep": np.ascontiguousarray(
            np.tile(ci_weight.astype(np.float32), (128, 1))),
        "gidx1": build_gidx1(configs),
        "gidx2": build_gidx2(configs),
    }


_CACHE: dict = {}


def _get_program():
    key = ("prog", BC, CT)
    if key not in _CACHE:
        _CACHE[key] = build(BC)
    return _CACHE[key]


def kernel(ao, mo_weight, ci_weight, configs):
    ao = np.asarray(ao, dtype=np.float32)
    mo_weight = np.asarray(mo_weight, dtype=np.float32)
    ci_weight = np.asarray(ci_weight, dtype=np.float32)
    configs = np.asarray(configs, dtype=np.int32)
    assert ao.shape == (B, NE, NAO)

    nc = _get_program()
    in_maps = [
        host_inputs(ao[c * BC:(c + 1) * BC], mo_weight, ci_weight, configs)
        for c in range(NCORES)
    ]
    res = run_bass_kernel_spmd(nc, in_maps, core_ids=list(range(NCORES)))
    outs = []
    for c in range(NCORES):
        o = np.asarray(res.results[c]["out"])      # [128, NBT]
        outs.append(o.T.reshape(-1))               # b = bt*128 + p
    return np.concatenate(outs).astype(np.float32)[:, None]


def ref_algo(ao_shard, mo_weight, ci_weight, configs):
    """Numpy replica of the on-device algorithm (dev checking only)."""
    M = np.einsum("ben,mn->bem", ao_shard, mo_weight[:NE]).astype(np.float32)
    sub = M[:, configs[:, :, None], configs[:, None, :]].astype(np.float32)
    subT = np.swapaxes(sub, -1, -2)
    Bs = subT.shape[0]
    A = subT.reshape(-1, K, K).copy()
    rcl = np.float32(RCLAMP)
    for k in range(K - 1):
        piv = A[:, k, k].copy()
        with np.errstate(divide="ignore"):
            rec = (np.float32(1.0) / piv).astype(np.float32)
        rec = np.clip(rec, -rcl, rcl)
        L = (A[:, k + 1:, k] * rec[:, None]).astype(np.float32)
        A[:, k + 1:, k + 1:] -= (
            L[:, :, None] * A[:, None, k, k + 1:].reshape(A.shape[0], 1, -1)
        ).astype(np.float32)
    diag = A[:, np.arange(K), np.arange(K)]
    t8 = diag[:, 0::2] * diag[:, 1::2]
    t4 = t8[:, 0::2] * t8[:, 1::2]
    t2 = t4[:, 0::2] * t4[:, 1::2]
    det = (t2[:, 0] * t2[:, 1]).astype(np.float32)
    dets_ = det.reshape(Bs, NCONF)
    return (dets_ @ ci_weight.T.astype(np.float32)).astype(np.float32)


# revision 4
# speedup vs baseline: 1.6431x; 1.0645x over previous
"""Trainium2 Bass kernel for nn_NEURAL_PYSCF_WF (neural wavefunction).

reference:
  mo   = einsum('ben,mn->bem', ao, mo_weight)          # [B, 32, 128]
  sub  = mo[:, cfg[:,:,None], cfg[:,None,:]]           # [B, 128, 16, 16]
  dets = det(sub)                                      # [B, 128]
  out  = dets @ ci_weight.T                            # [B, 1]

Config indices are < 32, so only mo[:, :, :32] matters.

Strategy (8 NeuronCores, data-parallel over B=8192). Per core (1024 rows):
  phase 1: ao tiles -> PE transpose -> matmul (W32T stationary) ->
           M^T [m,(b,e)] -> DRAM scratch; reload per 128-row b-tile as
           M [128b, 1024(e,m)].
  phase 2 per chunk of Ct configs:
           gather1 (GPSIMD ap_gather, d=16): row-halves of each config's
             16 rows -> R [b, c, i, m32]
           transpose copy (DVE, strided): R -> Rt [b, c, m32, i16]
           gather2 (d=16): config columns -> subT [b, c, j, i]
           pivot-free LU on DVE batched over (b partitions, configs in
           free dim); reciprocal clamped to +-1e6; det = prod(diag) via
           product tree (det(A^T) == det(A)).
  out[b] = sum_c ci[c] * det[b, c]  (TT mult + reduce).
"""

from contextlib import ExitStack

import numpy as np

import concourse.bass as bass
import concourse.bacc as bacc
import concourse.mybir as mybir
import concourse.tile as tile
from concourse.bass_utils import run_bass_kernel_spmd

F32 = mybir.dt.float32
I16 = mybir.dt.int16
AX = mybir.AxisListType
OP = mybir.AluOpType

B = 8192
NE = 32      # electrons (and the max config index)
NAO = 128
K = 16       # config size
NCONF = 128
NCORES = 8
BC = B // NCORES
RCLAMP = 1e6
CT = 16      # configs per chunk


def wrap_idx(idx: np.ndarray) -> np.ndarray:
    """Wrap a flat index list into ap_gather's [128, n/16] layout."""
    n = idx.shape[0]
    assert n % 16 == 0
    w = idx.reshape(n // 16, 16).T.astype(np.int16)
    return np.tile(w, (8, 1))


def build_gidx1(cfg: np.ndarray) -> np.ndarray:
    """Stage-1 indices: (c, i, h) -> block cfg[c,i]*2 + h (d=16 units)."""
    c = cfg.shape[0]
    idx = (cfg[:, :, None].astype(np.int64) * 2
           + np.arange(2)[None, None, :]).reshape(-1)
    return wrap_idx(idx)          # [128, NCONF*32/16]


def build_gidx2(cfg: np.ndarray) -> np.ndarray:
    """Stage-2 indices per chunk: (c_local, j) -> c_local*32 + cfg[c,j]."""
    nch = NCONF // CT
    cols = []
    for ch in range(nch):
        sl = cfg[ch * CT:(ch + 1) * CT]                    # [CT, 16]
        idx = (np.arange(CT)[:, None] * NE + sl).reshape(-1)
        cols.append(wrap_idx(idx))                         # [128, CT]
    return np.concatenate(cols, axis=1)                    # [128, NCONF]


def emit_program(nc, tc, aps, BCc: int):
    ctx = ExitStack()
    NBT = BCc // 128
    NCH = NCONF // CT
    ao, w32t, ident, cirep, gidx1, gidx2, mscr, out = (
        aps["ao"], aps["w32t"], aps["ident"], aps["cirep"], aps["gidx1"],
        aps["gidx2"], aps["mscr"], aps["out"])

    with ctx:
        cpool = ctx.enter_context(tc.tile_pool(name="consts", bufs=1))
        nat = ctx.enter_context(tc.tile_pool(name="nat", bufs=4))
        tp_ps = ctx.enter_context(
            tc.tile_pool(name="tp_ps", bufs=3, space="PSUM"))
        aot = ctx.enter_context(tc.tile_pool(name="aot", bufs=1))
        m_ps = ctx.enter_context(
            tc.tile_pool(name="m_ps", bufs=3, space="PSUM"))
        msb = ctx.enter_context(tc.tile_pool(name="msb", bufs=2))
        rp = ctx.enter_context(tc.tile_pool(name="rp", bufs=1))
        rtp = ctx.enter_context(tc.tile_pool(name="rtp", bufs=1))
        sub = ctx.enter_context(tc.tile_pool(name="sub", bufs=3))
        lb = ctx.enter_context(tc.tile_pool(name="lb", bufs=2))
        pb = ctx.enter_context(tc.tile_pool(name="pb", bufs=1))
        sm = ctx.enter_context(tc.tile_pool(name="sm", bufs=8))
        dets = ctx.enter_context(tc.tile_pool(name="dets", bufs=2))
        outp = ctx.enter_context(tc.tile_pool(name="outp", bufs=1))

        w32t_s = cpool.tile([128, NE], F32)
        ident_s = cpool.tile([128, 128], F32)
        cirep_s = cpool.tile([128, NCONF], F32)
        gidx1_s = cpool.tile([128, NCONF * 2], I16)
        gidx2_s = cpool.tile([128, NCONF], I16)
        nc.sync.dma_start(w32t_s[:], w32t[:])
        nc.sync.dma_start(ident_s[:], ident[:])
        nc.sync.dma_start(cirep_s[:], cirep[:])
        nc.sync.dma_start(gidx1_s[:], gidx1[:])
        nc.sync.dma_start(gidx2_s[:], gidx2[:])

        out_sb = outp.tile([128, NBT], F32)

        ao3 = ao.rearrange("(t p) n -> t p n", p=128)
        # mscr: [32m, BC*32(b,e)] — M^T layout
        mscr_r = mscr.rearrange("m (b e) -> b m e", e=NE)   # [BC, 32, 32]

        for bt in range(NBT):
            # ------------- phase 1: M^T = W32 @ ao^T -------------------
            aot_full = aot.tile([128, 32 * 128], F32)
            for t in range(32):
                nat_t = nat.tile([128, 128], F32)
                nc.sync.dma_start(nat_t[:], ao3[bt * 32 + t])
                ps = tp_ps.tile([128, 128], F32)
                nc.tensor.transpose(ps[:], nat_t[:], ident_s[:])
                nc.scalar.copy(aot_full[:, t * 128:(t + 1) * 128], ps[:])
            for t in range(32):
                mp = m_ps.tile([NE, 128], F32)
                nc.tensor.matmul(
                    mp[:], w32t_s[:], aot_full[:, t * 128:(t + 1) * 128],
                    start=True, stop=True)
                msb_s = nat.tile([NE, 128], F32, tag="mstage")
                nc.scalar.copy(msb_s[:], mp[:])
                nc.scalar.dma_start(
                    mscr[:, (bt * 128 + t * 4) * NE:
                         (bt * 128 + t * 4 + 4) * NE],
                    msb_s[:])

            # ------------- phase 2: dets -------------------------------
            msb_t = msb.tile([128, NE * NE], F32)
            nc.sync.dma_start(msb_t[:], mscr_r[bt * 128:(bt + 1) * 128])
            dets_t = dets.tile([128, NCONF], F32)
            for chp in range(NCH // 2):
                # gather a pair of chunks, then interleave their LUs so
                # the DVE scheduler can fill dependency bubbles.
                subs = []
                for cc in range(2):
                    ch = chp * 2 + cc
                    r_t = rp.tile([128, CT * K * NE], F32)
                    nc.gpsimd.ap_gather(
                        r_t[:], msb_t[:],
                        gidx1_s[:, ch * CT * 2:(ch + 1) * CT * 2],
                        channels=128, num_elems=NE * 2, d=16,
                        num_idxs=CT * K * 2)
                    # transpose [c, i, m] -> [c, m, i]
                    rt_t = rtp.tile([128, CT * K * NE], F32)
                    rt_dst = bass.AP(
                        rt_t[:].tensor, rt_t[:].offset,
                        [[int(rt_t[:].ap[0][0]), 128],
                         [K * NE, CT], [K, NE], [1, K]])
                    r_src = bass.AP(
                        r_t[:].tensor, r_t[:].offset,
                        [[int(r_t[:].ap[0][0]), 128],
                         [K * NE, CT], [1, NE], [NE, K]])
                    if cc == 0:
                        nc.vector.tensor_copy(rt_dst, r_src)
                    else:
                        nc.scalar.copy(rt_dst, r_src)
                    sub_t = sub.tile([128, CT * K * K], F32)
                    nc.gpsimd.ap_gather(
                        sub_t[:], rt_t[:],
                        gidx2_s[:, ch * CT:(ch + 1) * CT],
                        channels=128, num_elems=CT * NE, d=16,
                        num_idxs=CT * K)
                    subs.append(sub_t)

                # ---- two interleaved pivot-free LUs over [c, 16, 16] ----
                S4s, L3s, P4s = [], [], []
                for cc in range(2):
                    S4s.append(subs[cc][:].rearrange(
                        "p (c i j) -> p c i j", i=K, j=K))
                    lbuf = lb.tile([128, CT * (K - 1)], F32, tag=f"lb{cc}")
                    L3s.append(lbuf[:].rearrange("p (c i) -> p c i", c=CT))
                    pbuf = pb.tile(
                        [128, CT * (K - 1) * (K - 1)], F32, tag=f"pb{cc}")
                    P4s.append(pbuf[:].rearrange(
                        "p (c i j) -> p c i j", i=K - 1, j=K - 1))

                for k in range(K - 1):
                    r = K - 1 - k
                    for cc in range(2):
                        S4, L3, P4 = S4s[cc], L3s[cc], P4s[cc]
                        piv = S4[:, :, k, k]
                        rec = sm.tile([128, CT], F32, tag=f"rec{cc}")
                        nc.vector.reciprocal(rec[:], piv)
                        nc.vector.tensor_scalar(
                            rec[:], rec[:], -RCLAMP, RCLAMP,
                            op0=OP.max, op1=OP.min)
                        col = S4[:, :, k + 1:, k]
                        row = S4[:, :, k, k + 1:]
                        Lv = L3[:, :, :r]
                        nc.vector.tensor_tensor(
                            Lv, col,
                            rec[:].unsqueeze(2).broadcast_to([128, CT, r]),
                            op=OP.mult)
                        Pv = P4[:, :, :r, :r]
                        nc.vector.tensor_tensor(
                            Pv,
                            Lv.unsqueeze(3).broadcast_to([128, CT, r, r]),
                            row.unsqueeze(2).broadcast_to([128, CT, r, r]),
                            op=OP.mult)
                        Sv = S4[:, :, k + 1:, k + 1:]
                        nc.vector.tensor_tensor(Sv, Sv, Pv, op=OP.subtract)

                # det = prod(diag) via product tree
                for cc in range(2):
                    ch = chp * 2 + cc
                    diag = subs[cc][:]
                    t8 = sm.tile([128, CT * 8], F32, tag=f"t8{cc}")
                    nc.vector.tensor_tensor(
                        t8[:].rearrange("p (c x) -> p c x", c=CT),
                        bass.AP(diag.tensor, diag.offset,
                                [[int(diag.ap[0][0]), 128], [K * K, CT],
                                 [34, 8]]),
                        bass.AP(diag.tensor, diag.offset + 17,
                                [[int(diag.ap[0][0]), 128], [K * K, CT],
                                 [34, 8]]),
                        op=OP.mult)
                    t4 = sm.tile([128, CT * 4], F32, tag=f"t4{cc}")
                    nc.vector.tensor_tensor(
                        t4[:].rearrange("p (c x) -> p c x", c=CT),
                        bass.AP(t8[:].tensor, t8[:].offset,
                                [[int(t8[:].ap[0][0]), 128], [8, CT], [2, 4]]),
                        bass.AP(t8[:].tensor, t8[:].offset + 1,
                                [[int(t8[:].ap[0][0]), 128], [8, CT], [2, 4]]),
                        op=OP.mult)
                    t2 = sm.tile([128, CT * 2], F32, tag=f"t2{cc}")
                    nc.vector.tensor_tensor(
                        t2[:].rearrange("p (c x) -> p c x", c=CT),
                        bass.AP(t4[:].tensor, t4[:].offset,
                                [[int(t4[:].ap[0][0]), 128], [4, CT], [2, 2]]),
                        bass.AP(t4[:].tensor, t4[:].offset + 1,
                                [[int(t4[:].ap[0][0]), 128], [4, CT], [2, 2]]),
                        op=OP.mult)
                    nc.vector.tensor_tensor(
                        dets_t[:, ch * CT:(ch + 1) * CT],
                        bass.AP(t2[:].tensor, t2[:].offset,
                                [[int(t2[:].ap[0][0]), 128], [2, CT]]),
                        bass.AP(t2[:].tensor, t2[:].offset + 1,
                                [[int(t2[:].ap[0][0]), 128], [2, CT]]),
                        op=OP.mult)

            wd = sub.tile([128, NCONF], F32, tag="wd")
            nc.vector.tensor_tensor(wd[:], dets_t[:], cirep_s[:], op=OP.mult)
            nc.vector.tensor_reduce(
                out_sb[:, bt:bt + 1], wd[:], axis=AX.X, op=OP.add)

        nc.sync.dma_start(out[:], out_sb[:])


def build(BCc: int):
    nc = bacc.Bacc("TRN2", target_bir_lowering=False, debug=False)
    aps = {}
    aps["ao"] = nc.dram_tensor(
        "ao", [BCc * NE, NAO], F32, kind="ExternalInput").ap()
    aps["w32t"] = nc.dram_tensor(
        "w32t", [NAO, NE], F32, kind="ExternalInput").ap()
    aps["ident"] = nc.dram_tensor(
        "ident", [128, 128], F32, kind="ExternalInput").ap()
    aps["cirep"] = nc.dram_tensor(
        "cirep", [128, NCONF], F32, kind="ExternalInput").ap()
    aps["gidx1"] = nc.dram_tensor(
        "gidx1", [128, NCONF * 2], I16, kind="ExternalInput").ap()
    aps["gidx2"] = nc.dram_tensor(
        "gidx2", [128, NCONF], I16, kind="ExternalInput").ap()
    aps["mscr"] = nc.dram_tensor("mscr", [NE, BCc * NE], F32).ap()
    aps["out"] = nc.dram_tensor(
        "out", [128, BCc // 128], F32, kind="ExternalOutput").ap()

    with tile.TileContext(nc) as tc:
        emit_program(nc, tc, aps, BCc)
    nc.compile()
    return nc


def host_inputs(ao_shard, mo_weight, ci_weight, configs):
    BCc = ao_shard.shape[0]
    w32 = mo_weight[:NE, :]
    return {
        "ao": np.ascontiguousarray(
            ao_shard.reshape(BCc * NE, NAO)).astype(np.float32),
        "w32t": np.ascontiguousarray(w32.T).astype(np.float32),
        "ident": np.eye(128, dtype=np.float32),
        "cirep": np.ascontiguousarray(
            np.tile(ci_weight.astype(np.float32), (128, 1))),
        "gidx1": build_gidx1(configs),
        "gidx2": build_gidx2(configs),
    }


_CACHE: dict = {}


def _get_program():
    key = ("prog", BC, CT)
    if key not in _CACHE:
        _CACHE[key] = build(BC)
    return _CACHE[key]


def kernel(ao, mo_weight, ci_weight, configs):
    ao = np.asarray(ao, dtype=np.float32)
    mo_weight = np.asarray(mo_weight, dtype=np.float32)
    ci_weight = np.asarray(ci_weight, dtype=np.float32)
    configs = np.asarray(configs, dtype=np.int32)
    assert ao.shape == (B, NE, NAO)

    nc = _get_program()
    in_maps = [
        host_inputs(ao[c * BC:(c + 1) * BC], mo_weight, ci_weight, configs)
        for c in range(NCORES)
    ]
    res = run_bass_kernel_spmd(nc, in_maps, core_ids=list(range(NCORES)))
    outs = []
    for c in range(NCORES):
        o = np.asarray(res.results[c]["out"])      # [128, NBT]
        outs.append(o.T.reshape(-1))               # b = bt*128 + p
    return np.concatenate(outs).astype(np.float32)[:, None]


def ref_algo(ao_shard, mo_weight, ci_weight, configs):
    """Numpy replica of the on-device algorithm (dev checking only)."""
    M = np.einsum("ben,mn->bem", ao_shard, mo_weight[:NE]).astype(np.float32)
    sub = M[:, configs[:, :, None], configs[:, None, :]].astype(np.float32)
    subT = np.swapaxes(sub, -1, -2)
    Bs = subT.shape[0]
    A = subT.reshape(-1, K, K).copy()
    rcl = np.float32(RCLAMP)
    for k in range(K - 1):
        piv = A[:, k, k].copy()
        with np.errstate(divide="ignore"):
            rec = (np.float32(1.0) / piv).astype(np.float32)
        rec = np.clip(rec, -rcl, rcl)
        L = (A[:, k + 1:, k] * rec[:, None]).astype(np.float32)
        A[:, k + 1:, k + 1:] -= (
            L[:, :, None] * A[:, None, k, k + 1:].reshape(A.shape[0], 1, -1)
        ).astype(np.float32)
    diag = A[:, np.arange(K), np.arange(K)]
    t8 = diag[:, 0::2] * diag[:, 1::2]
    t4 = t8[:, 0::2] * t8[:, 1::2]
    t2 = t4[:, 0::2] * t4[:, 1::2]
    det = (t2[:, 0] * t2[:, 1]).astype(np.float32)
    dets_ = det.reshape(Bs, NCONF)
    return (dets_ @ ci_weight.T.astype(np.float32)).astype(np.float32)
